# revision 11
# baseline (speedup 1.0000x reference)
"""Condensation loss (Tiger) on 8 Trainium2 NeuronCores.

Strategy (per sharding hint): shard the hit dimension N across 8 cores,
replicate the K-1 condensation points, assemble the scalar loss on host.

Math restructure vs the naive kernel: the repulsive term
  v_rep = sum_{n,k} q_n q_k (1 - dist_nk) [dist_nk < 1][~att]
is nonzero only for pairs with d2 < 1.  The device computes the full
N x K d2 matrix on the PE (bf16 inputs, fp32 PSUM) and reduces each row
to a tiny *detector* output instead of evaluating sqrt/min per element:
  - DVE lane:  tensor_tensor_reduce(min, min) over the tile's two
               512-col PSUM halves -> rowmin of d2 in ~703ns (the
               elementwise stage reads both halves concurrently, halving
               the 1-elem/cycle/lane fp32 PSUM drain cost)
  - ACT lane:  activation(Relu, scale=-1, bias=4) + accum_out
               -> rowsum of relu(4 - d2)
A row can contain a d2 < 1 pair only if its detector fires (bf16 input
rounding shifts d2 by well under the 2.5 flag margin; a guard falls back
to flagging everything for out-of-range inputs).  The host recomputes
flagged rows exactly in fp64 (~1k rows: the condensation points
themselves plus hit 0 for empty objects).  v_att (O(N*D)), l_noise and
l_coward are exact on host in fp64.

Device layout per core: 6272 padded hits = 49 row-tiles of 128 (covers
the 6250 real rows).  Each tile's d2 [128, 1024] lives in one 2-bank
PSUM tile; 4 buffers fill all 8 banks.  Even tiles compute at PE array
rows 0:34, odd tiles at 64:98 (tile_position row-packing) so consecutive
tiles' LDWEIGHTS/MATMULs overlap; the PE streams 1024 cols/tile at
2.4GHz (427ns) and is the steady-state floor.  Tiles are assigned
cyclically [DVE, DVE, ACT]: DVE at ~703ns/tile and ACT at ~1313ns/tile
both keep up with the PE.  Input DMAs issue in parallel on the SP and
ACT HWDGE queues plus the GpSimd SWDGE queue so the first matmul starts
as early as the framework preamble allows.
"""

import os
import numpy as np
import ml_dtypes

# ---------------- geometry (hardcoded per the task contract) ----------------
N_HITS = 50000
D_EMB = 32
N_CLUSTERS = 1024          # ids 0..1023; objects are 1..1023
N_OBJ = N_CLUSTERS - 1     # 1023
KP = 1024                  # padded object columns (col j = object j+1; col 1023 dummy)
NCORES = 8
N_PER = N_HITS // NCORES   # 6250
NT = 49                    # row tiles per core (49*128 = 6272 >= 6250)
NPAIR = 25                 # xt packing blocks (tile pairs; block 24 odd half unused)
CDIM = D_EMB + 2           # contraction: [x(32), r2, 1]
THR = 4.0                  # detector threshold on d2 (flag margin vs dist<1)

Q_MIN = 0.01
PT_THLD = 0.9
MAX_ETA = 4.0
LW_REP = 1.0
LW_NOISE = 0.1
LW_COWARD = 0.1
EPS = 1e-9

_BF16 = ml_dtypes.bfloat16

_STATE = {}


DVE_COST = 1223.0          # ns per [128,1024] fp32 PSUM tensor_reduce (measured)
ACT_COST = 1271.0          # ns per tile on ACT incl READ_ACC overlap (measured)


def _tile_split():
    """Greedy least-loaded assignment of the tile drains to DVE/ACT."""
    dve, act = [], []
    lv = la = 0.0
    for t in range(NT):
        if lv + DVE_COST <= la + ACT_COST:
            dve.append(t)
            lv += DVE_COST
        else:
            act.append(t)
            la += ACT_COST
    return dve, act


# ---------------- device module ----------------
def _build_module():
    import concourse.bacc as bacc
    import concourse.mybir as mybir
    import concourse.tile as tile
    from contextlib import ExitStack

    dve_tiles, act_tiles = _tile_split()
    nv, na = len(dve_tiles), len(act_tiles)
    lane = {}
    for i, t in enumerate(dve_tiles):
        lane[t] = ("V", i)
    for i, t in enumerate(act_tiles):
        lane[t] = ("A", nv + i)

    nc = bacc.Bacc("TRN2", target_bir_lowering=False, debug=False,
                   num_devices=NCORES)
    dt = mybir.dt

    xt_d = nc.dram_tensor("xt", [128, NPAIR * 128], dt.bfloat16,
                          kind="ExternalInput").ap()
    xkt_d = nc.dram_tensor("xkt", [128, KP], dt.bfloat16,
                           kind="ExternalInput").ap()
    det_d = nc.dram_tensor("det_out", [128, nv + na], dt.float32,
                           kind="ExternalOutput").ap()

    with tile.TileContext(nc) as tc, ExitStack() as ctx:
        consts = ctx.enter_context(tc.tile_pool(name="consts", bufs=1))
        scra_p = ctx.enter_context(tc.tile_pool(name="scra", bufs=2))
        psum = ctx.enter_context(tc.tile_pool(name="psum", bufs=4, space="PSUM"))

        # Input DMAs fan out across three queues (SP, ACT HWDGE + GpSimd
        # SWDGE) so issue (~600ns each on HWDGE, ~1us engine on SWDGE)
        # overlaps; the first matmul's operands (xt block 0 on SP, xkt
        # halves on ACT) issue first on their queues.  After the tile
        # context closes, these instructions are hoisted into the `main`
        # preamble block so the transfers overlap the framework barrier.
        xkt_sb = consts.tile([128, KP], dt.bfloat16)
        xt_sb = consts.tile([128, NPAIR * 128], dt.bfloat16)
        # Small critical chunks (xt block 0 + xkt quarters, 64KB each)
        # lead their queues and get hoisted into the preamble: the
        # barrier's queue-drain then stalls only ~1.5us on them while the
        # first matmuls' operands land before the tile context even opens.
        nc.sync.dma_start(out=xt_sb[:, 0:256], in_=xt_d[:, 0:256])
        nc.scalar.dma_start(out=xkt_sb[:, 0:256], in_=xkt_d[:, 0:256])
        nc.scalar.dma_start(out=xkt_sb[:, 256:512], in_=xkt_d[:, 256:512])
        nc.sync.dma_start(out=xkt_sb[:, 512:768], in_=xkt_d[:, 512:768])
        nc.sync.dma_start(out=xkt_sb[:, 768:1024], in_=xkt_d[:, 768:1024])
        # bulk xt stays in the tile context (issued right after the barrier)
        nc.scalar.dma_start(out=xt_sb[:, 256:768], in_=xt_d[:, 256:768])
        nc.sync.dma_start(out=xt_sb[:, 768:1664], in_=xt_d[:, 768:1664])
        nc.gpsimd.dma_start(out=xt_sb[:, 1664:2432], in_=xt_d[:, 1664:2432])
        nc.gpsimd.dma_start(out=xt_sb[:, 2432:3200], in_=xt_d[:, 2432:3200])
        thrb_sb = consts.tile([128, 1], dt.float32)
        nc.gpsimd.memset(thrb_sb, THR)
        det_sb = consts.tile([128, nv + na], dt.float32)

        for t in range(NT):
            # even tiles live at PE array rows 0:34, odd tiles at 64:98 —
            # consecutive tiles' LDWEIGHTS/MATMULs overlap (per-subarray
            # concurrency), and the 4 psum buffers keep both drain engines
            # independently buffered ahead of the PE.
            p, base = t // 2, (0 if t % 2 == 0 else 64)
            ps = psum.tile([128, 1024], dt.float32, tag="d2")
            lhs = xt_sb[base:base + CDIM, p * 128:(p + 1) * 128]
            nc.tensor.matmul(ps[:, 0:512], lhs, xkt_sb[base:base + CDIM, 0:512],
                             start=True, stop=True, tile_position=(base, 0))
            nc.tensor.matmul(ps[:, 512:1024], lhs,
                             xkt_sb[base:base + CDIM, 512:1024],
                             start=True, stop=True, tile_position=(base, 0))
            which, idx = lane[t]
            if which == "V":
                nc.vector.tensor_reduce(det_sb[:, idx:idx + 1], ps,
                                        axis=mybir.AxisListType.X,
                                        op=mybir.AluOpType.min)
            else:
                scr = scra_p.tile([128, 1024], dt.bfloat16, tag="scra")
                nc.scalar.activation(
                    scr, ps, mybir.ActivationFunctionType.Relu,
                    bias=thrb_sb, scale=-1.0,
                    accum_out=det_sb[:, idx:idx + 1])
            # early partial drains of finished detector columns keep the
            # final output DMAs tiny (tail = last few columns only)
            if t == NT - 12:
                ev = sum(1 for tt in dve_tiles if tt <= t)
                ea = sum(1 for tt in act_tiles if tt <= t)
                nc.sync.dma_start(out=det_d[:, 0:ev], in_=det_sb[:, 0:ev])
                nc.sync.dma_start(out=det_d[:, nv:nv + ea],
                                  in_=det_sb[:, nv:nv + ea])
                early = (ev, ea)

        # remaining detector columns: DVE block on SP as soon as the last
        # V-unit is done, ACT block on the ACT queue right after its own
        # last READ_ACC (the two issues overlap)
        ev, ea = early
        nc.sync.dma_start(out=det_d[:, ev:nv], in_=det_sb[:, ev:nv])
        nc.scalar.dma_start(out=det_d[:, nv + ea:], in_=det_sb[:, nv + ea:])

    # Hoist the (wait-free) input DMA instructions from the tile-context
    # block into the `main` preamble block, ahead of the all-engine
    # barrier: each engine issues its input DMAs the moment its stream
    # starts, so the HBM transfers run under the framework's startup
    # rendezvous instead of after it.
    if os.environ.get("COND_HOIST", "1") == "1":
        f = nc.m.functions[0]
        main_bb = f.blocks[0]
        tc_bb = next(b for b in f.blocks if b.name.startswith("tile_context"))

        def _elems(inst):
            n = 1
            for _, cnt in inst.ins[0].ap:
                n *= cnt
            return n

        hoist = []
        for inst in list(tc_bb.instructions):
            if type(inst).__name__ == "InstDMACopy" and \
                    str(inst.ins[0].memref) in ("xt", "xkt") and \
                    _elems(inst) <= 128 * 256:
                si = inst.sync_info
                assert si is None or not si.on_wait, \
                    f"input DMA {inst.name} grew a wait; hoist unsafe"
                hoist.append(inst)
        assert len(hoist) == 5, f"expected 5 small input DMAs, {len(hoist)}"
        hset = set(id(i) for i in hoist)
        tc_bb.instructions = [i for i in tc_bb.instructions
                              if id(i) not in hset]
        mb = list(main_bb.instructions)
        main_bb.instructions = mb[:1] + hoist + mb[1:]

    nc.compile()
    return nc


def _get_module():
    if "nc" not in _STATE:
        _STATE["nc"] = _build_module()
    return _STATE["nc"]


# ---------------- host prep ----------------
def _prep(beta, x, pt, eta, reconstructable, cluster_ids):
    f32 = np.float32
    beta = np.asarray(beta, f32)
    x = np.ascontiguousarray(np.asarray(x, f32))
    pt = np.asarray(pt, f32)
    eta = np.asarray(eta, f32)
    recon = np.asarray(reconstructable)
    cid = np.asarray(cluster_ids).astype(np.int64)

    # alpha selection in fp32 to match the reference's argmax semantics
    q32 = (np.arctanh(np.clip(beta, 0.0, 1.0 - 1e-4)) ** 2 + Q_MIN).astype(f32)
    hit_ok = (recon > 0) & (pt > PT_THLD) & (np.abs(eta) < MAX_ETA)
    cid_eff = np.where(hit_ok, cid, 0)
    best = np.zeros(N_CLUSTERS, f32)
    np.maximum.at(best, cid_eff, q32)
    idx = np.full(N_CLUSTERS, N_HITS, np.int64)
    ismax = (q32 == best[cid_eff]) & (cid_eff > 0)
    np.minimum.at(idx, cid_eff[ismax], np.nonzero(ismax)[0])
    alphas = np.where(idx[1:] < N_HITS, idx[1:], 0)      # [1023]

    # device operands: bf16-quantized hits + condensation points
    xq = x.astype(_BF16)                                 # [N, 32]
    xqf = xq.astype(f32)
    r2q = np.einsum('nd,nd->n', xqf, xqf).astype(f32)
    r2b = r2q.astype(_BF16)

    NPC = NPAIR * 256                                    # padded rows/core (6400)
    X34 = np.zeros((NCORES * NPC, CDIM), f32)
    real = np.zeros(NCORES * NPC, bool)
    for c in range(NCORES):
        real[c * NPC:c * NPC + N_PER] = True
    X34[real, :D_EMB] = xqf
    X34[real, D_EMB] = r2b.astype(f32)
    X34[:, D_EMB + 1] = 1.0
    X34 = X34.astype(_BF16)

    Y34 = np.zeros((KP, CDIM), f32)
    Y34[:N_OBJ, :D_EMB] = -2.0 * xqf[alphas]
    Y34[:N_OBJ, D_EMB] = 1.0
    Y34[:N_OBJ, D_EMB + 1] = r2b[alphas].astype(f32)
    Y34[N_OBJ] = 0.0
    Y34[N_OBJ, D_EMB] = 1.0
    Y34[N_OBJ, D_EMB + 1] = 1e4                          # dummy far column
    Y34 = Y34.astype(_BF16)
    xkt = np.zeros((128, KP), _BF16)
    xkt[0:CDIM] = Y34.T
    xkt[64:64 + CDIM] = Y34.T

    in_maps = []
    for c in range(NCORES):
        A = X34[c * NPC:(c + 1) * NPC].reshape(2 * NPAIR, 128, CDIM
                                               ).transpose(0, 2, 1)
        xt_c = np.zeros((128, NPAIR * 128), _BF16)
        xt_c[0:CDIM] = A[0::2].transpose(1, 0, 2).reshape(CDIM, NPAIR * 128)
        xt_c[64:64 + CDIM] = A[1::2].transpose(1, 0, 2).reshape(
            CDIM, NPAIR * 128)
        in_maps.append({"xt": np.ascontiguousarray(xt_c), "xkt": xkt})

    aux = dict(q32=q32, hit_ok=hit_ok, cid=cid, beta=beta, x=x,
               alphas=alphas)
    return in_maps, aux


# ---------------- host finish ----------------
def _finish(results, aux):
    f64 = np.float64
    q32, alphas = aux["q32"], aux["alphas"]
    hit_ok, cid, beta, x = aux["hit_ok"], aux["cid"], aux["beta"], aux["x"]

    q = q32.astype(f64)
    x64 = x.astype(f64)
    xk64 = x64[alphas]                                   # [1023, 32]
    qk = q[alphas]

    dve_tiles, act_tiles = _tile_split()
    nv = len(dve_tiles)

    # ---- gather flagged hits from the detectors ----
    flagged = set()
    for c in range(NCORES):
        det = np.asarray(results[c]["det_out"], f64)     # [128, nv+na]
        fl = np.zeros((128, NT), bool)
        fl[:, dve_tiles] = det[:, :nv] < THR - 0.5
        fl[:, act_tiles] = det[:, nv:] > 0.45
        rr, tt = np.nonzero(fl)
        for r, t in zip(rr, tt):
            n = t * 128 + r
            if n < N_PER:
                flagged.add(c * N_PER + n)
    flagged = np.fromiter(sorted(flagged), dtype=np.int64,
                          count=len(flagged))

    # safety: the detector's bf16 error margin assumes moderate |x|; the
    # dominant term is the bf16 rounding of |x|^2, so bound that directly
    if (not np.isfinite(x).all()) or \
            float(np.einsum('nd,nd->n', x64, x64).max()) > 200.0:
        flagged = np.arange(N_HITS, dtype=np.int64)
    if os.environ.get("COND_KERNEL_DEBUG", "0") == "1":
        print(f"[kernel] flagged rows: {len(flagged)}")

    # ---- v_rep: exact fp64 over flagged rows only ----
    v_rep_num = 0.0
    if len(flagged):
        xf = x64[flagged]
        d2 = (np.einsum('nd,nd->n', xf, xf)[:, None]
              + np.einsum('kd,kd->k', xk64, xk64)[None, :]
              - 2.0 * (xf @ xk64.T))
        dist = np.sqrt(np.maximum(d2, 1e-12))
        att = (cid[flagged][:, None] == np.arange(1, N_CLUSTERS)[None, :]) \
            & hit_ok[flagged][:, None]
        rep = (~att) & (dist < 1.0)
        qw = q[flagged][:, None] * qk[None, :]
        v_rep_num = float(np.sum(qw * (1.0 - dist) * rep))

    # ---- v_att: exact fp64 on the attractive pairs ----
    att_idx = np.nonzero(hit_ok & (cid > 0))[0]
    kk = cid[att_idx] - 1
    diff = x64[att_idx] - xk64[kk]
    d2a = np.maximum(np.einsum('nd,nd->n', diff, diff), 1e-12)
    v_att_num = float(np.sum(q[att_idx] * qk[kk] * d2a))

    n_hits_oi = float(hit_ok.sum())
    norm_att = EPS + n_hits_oi - N_OBJ
    norm_rep = EPS + (N_OBJ - 1) * N_HITS
    v_att = v_att_num / norm_att
    v_rep = v_rep_num / norm_rep

    noise_mask = (cid <= 0)
    l_noise = float(beta[noise_mask].sum()) / max(float(noise_mask.sum()), 1.0)
    l_coward = float(np.mean(1.0 - beta[alphas]))

    total = v_att + LW_REP * v_rep + LW_NOISE * l_noise + LW_COWARD * l_coward
    return np.asarray(total, dtype=np.float32)


# ---------------- execution backends ----------------
def _run_sim(nc, in_maps):
    from concourse.bass_interp import CoreSim
    results = []
    for m in in_maps:
        sim = CoreSim(nc)
        for k, v in m.items():
            sim.tensor(k)[:] = v
        sim.simulate()
        results.append({k: np.array(sim.tensor(k))
                        for k in ("det_out",)})
    return results


def _ensure_ntff_hook():
    """Register the axon NTFF profiling hook if the antenv shim lacks it."""
    import sys
    import types
    try:
        from antenv.axon_hooks import get_axon_ntff_profile_hook  # noqa: F401
        return
    except ImportError:
        pass
    from trn_agent_boot.trn_boot import _ntff_profile_via_ctypes
    hook = _ntff_profile_via_ctypes("/opt/axon/libaxon_pjrt.so")
    mod = types.ModuleType("antenv.axon_hooks")
    _h = [hook]
    mod.set_axon_ntff_profile_hook = lambda h: _h.__setitem__(0, h)
    mod.get_axon_ntff_profile_hook = lambda: _h[0]
    sys.modules["antenv.axon_hooks"] = mod
    import antenv
    antenv.axon_hooks = mod


def _run_hw(nc, in_maps, trace=False):
    import tempfile
    from concourse.bass_utils import run_bass_kernel_spmd
    core_ids = list(range(NCORES))
    if trace:
        try:
            _ensure_ntff_hook()
            tmpdir = tempfile.mkdtemp(prefix="cond_trace_")
            res = run_bass_kernel_spmd(nc, in_maps, core_ids, trace=True,
                                       tmpdir=tmpdir)
            _STATE["last_exec_time_ns"] = res.exec_time_ns
            _STATE["last_trace_dir"] = tmpdir
            _STATE["last_profile_json"] = res.profile_json
            return res.results
        except Exception:
            import traceback
            traceback.print_exc()
            print("[kernel] traced run failed; retrying without trace")
    res = run_bass_kernel_spmd(nc, in_maps, core_ids, trace=False)
    _STATE["last_exec_time_ns"] = res.exec_time_ns
    return res.results


def kernel(beta, x, pt, eta, reconstructable, cluster_ids, n_clusters=None,
           **_ignored):
    in_maps, aux = _prep(beta, x, pt, eta, reconstructable, cluster_ids)
    nc = _get_module()
    if os.environ.get("COND_KERNEL_SIM", "0") == "1":
        results = _run_sim(nc, in_maps)
    else:
        results = _run_hw(nc, in_maps,
                          trace=os.environ.get("COND_KERNEL_TRACE", "0") == "1")
    return _finish(results, aux)


# revision 14
# speedup vs baseline: 1.0507x; 1.0507x over previous
"""Condensation loss (Tiger) on 8 Trainium2 NeuronCores.

Strategy (per sharding hint): shard the hit dimension N across 8 cores,
replicate the K-1 condensation points, assemble the scalar loss on host.

Math restructure vs the naive kernel: the repulsive term
  v_rep = sum_{n,k} q_n q_k (1 - dist_nk) [dist_nk < 1][~att]
is nonzero only for pairs with d2 < 1.  The device computes the full
N x K d2 matrix on the PE (bf16 inputs, fp32 PSUM) and reduces each row
to a tiny *detector* output instead of evaluating sqrt/min per element:
  - DVE lane:  tensor_tensor_reduce(min, min) over the tile's two
               512-col PSUM halves -> rowmin of d2 in ~703ns (the
               elementwise stage reads both halves concurrently, halving
               the 1-elem/cycle/lane fp32 PSUM drain cost)
  - ACT lane:  activation(Relu, scale=-1, bias=4) + accum_out
               -> rowsum of relu(4 - d2)
A row can contain a d2 < 1 pair only if its detector fires (bf16 input
rounding shifts d2 by well under the 2.5 flag margin; a guard falls back
to flagging everything for out-of-range inputs).  The host recomputes
flagged rows exactly in fp64 (~1k rows: the condensation points
themselves plus hit 0 for empty objects).  v_att (O(N*D)), l_noise and
l_coward are exact on host in fp64.

Device layout per core: 6272 padded hits = 49 row-tiles of 128 (covers
the 6250 real rows).  Each tile's d2 [128, 1024] lives in one 2-bank
PSUM tile; 4 buffers fill all 8 banks.  Even tiles compute at PE array
rows 0:34, odd tiles at 64:98 (tile_position row-packing) so consecutive
tiles' LDWEIGHTS/MATMULs overlap; the PE streams 1024 cols/tile at
2.4GHz (427ns) and is the steady-state floor.  Tiles are assigned
cyclically [DVE, DVE, ACT]: DVE at ~703ns/tile and ACT at ~1313ns/tile
both keep up with the PE.  Input DMAs issue in parallel on the SP and
ACT HWDGE queues plus the GpSimd SWDGE queue so the first matmul starts
as early as the framework preamble allows.
"""

import os
import numpy as np
import ml_dtypes

# ---------------- geometry (hardcoded per the task contract) ----------------
N_HITS = 50000
D_EMB = 32
N_CLUSTERS = 1024          # ids 0..1023; objects are 1..1023
N_OBJ = N_CLUSTERS - 1     # 1023
KP = 1024                  # padded object columns (col j = object j+1; col 1023 dummy)
NCORES = 8
N_PER = N_HITS // NCORES   # 6250
NT = 49                    # row tiles per core (49*128 = 6272 >= 6250)
NPAIR = 25                 # xt packing blocks (tile pairs; block 24 odd half unused)
CDIM = D_EMB + 2           # contraction: [x(32), r2, 1]
THR = 4.0                  # detector threshold on d2 (flag margin vs dist<1)

Q_MIN = 0.01
PT_THLD = 0.9
MAX_ETA = 4.0
LW_REP = 1.0
LW_NOISE = 0.1
LW_COWARD = 0.1
EPS = 1e-9

_BF16 = ml_dtypes.bfloat16

_STATE = {}


def _tile_split():
    """Strict alternation: even tiles -> DVE (25), odd -> ACT (24).

    Measured per-tile drain costs are ~1223ns (DVE tensor_reduce) and
    ~1271ns (ACT activation+READ_ACC with overlap), so 25/24 is the
    balanced split; alternation also starts both engines immediately and
    spreads the PSUM-buffer hold times evenly."""
    dve = [t for t in range(NT) if t % 2 == 0]
    act = [t for t in range(NT) if t % 2 == 1]
    return dve, act


# ---------------- device module ----------------
def _build_module():
    import concourse.bacc as bacc
    import concourse.mybir as mybir
    import concourse.tile as tile
    from contextlib import ExitStack

    dve_tiles, act_tiles = _tile_split()
    nv, na = len(dve_tiles), len(act_tiles)
    lane = {}
    for i, t in enumerate(dve_tiles):
        lane[t] = ("V", i)
    for i, t in enumerate(act_tiles):
        lane[t] = ("A", nv + i)

    nc = bacc.Bacc("TRN2", target_bir_lowering=False, debug=False,
                   num_devices=NCORES)
    dt = mybir.dt

    xt_d = nc.dram_tensor("xt", [128, NPAIR * 128], dt.bfloat16,
                          kind="ExternalInput").ap()
    xkt_d = nc.dram_tensor("xkt", [128, KP], dt.bfloat16,
                           kind="ExternalInput").ap()
    det_d = nc.dram_tensor("det_out", [128, nv + na], dt.float32,
                           kind="ExternalOutput").ap()

    with tile.TileContext(nc) as tc, ExitStack() as ctx:
        consts = ctx.enter_context(tc.tile_pool(name="consts", bufs=1))
        scra_p = ctx.enter_context(tc.tile_pool(name="scra", bufs=2))
        psum = ctx.enter_context(tc.tile_pool(name="psum", bufs=4, space="PSUM"))

        # Input DMAs fan out across three queues (SP, ACT HWDGE + GpSimd
        # SWDGE) so issue (~600ns each on HWDGE, ~1us engine on SWDGE)
        # overlaps; the first matmul's operands (xt block 0 on SP, xkt
        # halves on ACT) issue first on their queues.  After the tile
        # context closes, these instructions are hoisted into the `main`
        # preamble block so the transfers overlap the framework barrier.
        xkt_sb = consts.tile([128, KP], dt.bfloat16)
        xt_sb = consts.tile([128, NPAIR * 128], dt.bfloat16)
        # Small critical chunks (xt block 0 + xkt quarters, 64KB each)
        # lead their queues and get hoisted into the preamble: the
        # barrier's queue-drain then stalls only ~1.5us on them while the
        # first matmuls' operands land before the tile context even opens.
        nc.sync.dma_start(out=xt_sb[:, 0:256], in_=xt_d[:, 0:256])
        nc.scalar.dma_start(out=xkt_sb[:, 0:256], in_=xkt_d[:, 0:256])
        nc.scalar.dma_start(out=xkt_sb[:, 256:512], in_=xkt_d[:, 256:512])
        nc.sync.dma_start(out=xkt_sb[:, 512:768], in_=xkt_d[:, 512:768])
        nc.sync.dma_start(out=xkt_sb[:, 768:1024], in_=xkt_d[:, 768:1024])
        # bulk xt stays in the tile context (issued right after the
        # barrier); the chunk feeding tiles 4-11 leads the SP queue so the
        # PE never starves after the hoisted block-0 chunk runs out
        nc.sync.dma_start(out=xt_sb[:, 256:768], in_=xt_d[:, 256:768])
        nc.sync.dma_start(out=xt_sb[:, 768:1664], in_=xt_d[:, 768:1664])
        nc.gpsimd.dma_start(out=xt_sb[:, 1664:2432], in_=xt_d[:, 1664:2432])
        nc.gpsimd.dma_start(out=xt_sb[:, 2432:3200], in_=xt_d[:, 2432:3200])
        thrb_sb = consts.tile([128, 1], dt.float32)
        nc.gpsimd.memset(thrb_sb, THR)
        det_sb = consts.tile([128, nv + na], dt.float32)

        for t in range(NT):
            # even tiles live at PE array rows 0:34, odd tiles at 64:98 —
            # consecutive tiles' LDWEIGHTS/MATMULs overlap (per-subarray
            # concurrency), and the 4 psum buffers keep both drain engines
            # independently buffered ahead of the PE.
            p, base = t // 2, (0 if t % 2 == 0 else 64)
            ps = psum.tile([128, 1024], dt.float32, tag="d2")
            lhs = xt_sb[base:base + CDIM, p * 128:(p + 1) * 128]
            nc.tensor.matmul(ps[:, 0:512], lhs, xkt_sb[base:base + CDIM, 0:512],
                             start=True, stop=True, tile_position=(base, 0))
            nc.tensor.matmul(ps[:, 512:1024], lhs,
                             xkt_sb[base:base + CDIM, 512:1024],
                             start=True, stop=True, tile_position=(base, 0))
            which, idx = lane[t]
            if which == "V":
                nc.vector.tensor_reduce(det_sb[:, idx:idx + 1], ps,
                                        axis=mybir.AxisListType.X,
                                        op=mybir.AluOpType.min)
            else:
                scr = scra_p.tile([128, 1024], dt.bfloat16, tag="scra")
                nc.scalar.activation(
                    scr, ps, mybir.ActivationFunctionType.Relu,
                    bias=thrb_sb, scale=-1.0,
                    accum_out=det_sb[:, idx:idx + 1])
            # early partial drains of finished detector columns keep the
            # final output DMAs tiny (tail = last few columns only)
            if t in (NT - 12, NT - 3):
                ev = sum(1 for tt in dve_tiles if tt <= t)
                ea = sum(1 for tt in act_tiles if tt <= t)
                pv, pa = early if t == NT - 3 else (0, 0)
                nc.sync.dma_start(out=det_d[:, pv:ev], in_=det_sb[:, pv:ev])
                nc.sync.dma_start(out=det_d[:, nv + pa:nv + ea],
                                  in_=det_sb[:, nv + pa:nv + ea])
                early = (ev, ea)

        # remaining detector columns: DVE block on SP as soon as the last
        # V-unit is done, ACT block on the ACT queue right after its own
        # last READ_ACC (the two issues overlap)
        ev, ea = early
        nc.sync.dma_start(out=det_d[:, ev:nv], in_=det_sb[:, ev:nv])
        nc.scalar.dma_start(out=det_d[:, nv + ea:], in_=det_sb[:, nv + ea:])

    # Hoist the (wait-free) input DMA instructions from the tile-context
    # block into the `main` preamble block, ahead of the all-engine
    # barrier: each engine issues its input DMAs the moment its stream
    # starts, so the HBM transfers run under the framework's startup
    # rendezvous instead of after it.
    if os.environ.get("COND_HOIST", "1") == "1":
        f = nc.m.functions[0]
        main_bb = f.blocks[0]
        tc_bb = next(b for b in f.blocks if b.name.startswith("tile_context"))

        def _elems(inst):
            n = 1
            for _, cnt in inst.ins[0].ap:
                n *= cnt
            return n

        hoist = []
        for inst in list(tc_bb.instructions):
            if type(inst).__name__ == "InstDMACopy" and \
                    str(inst.ins[0].memref) in ("xt", "xkt") and \
                    _elems(inst) <= 128 * 256:
                si = inst.sync_info
                assert si is None or not si.on_wait, \
                    f"input DMA {inst.name} grew a wait; hoist unsafe"
                hoist.append(inst)
        assert len(hoist) == 5, f"expected 5 small input DMAs, {len(hoist)}"
        hset = set(id(i) for i in hoist)
        tc_bb.instructions = [i for i in tc_bb.instructions
                              if id(i) not in hset]
        mb = list(main_bb.instructions)
        main_bb.instructions = mb[:1] + hoist + mb[1:]

    nc.compile()
    return nc


def _get_module():
    if "nc" not in _STATE:
        _STATE["nc"] = _build_module()
    return _STATE["nc"]


# ---------------- host prep ----------------
def _prep(beta, x, pt, eta, reconstructable, cluster_ids):
    f32 = np.float32
    beta = np.asarray(beta, f32)
    x = np.ascontiguousarray(np.asarray(x, f32))
    pt = np.asarray(pt, f32)
    eta = np.asarray(eta, f32)
    recon = np.asarray(reconstructable)
    cid = np.asarray(cluster_ids).astype(np.int64)

    # alpha selection in fp32 to match the reference's argmax semantics
    q32 = (np.arctanh(np.clip(beta, 0.0, 1.0 - 1e-4)) ** 2 + Q_MIN).astype(f32)
    hit_ok = (recon > 0) & (pt > PT_THLD) & (np.abs(eta) < MAX_ETA)
    cid_eff = np.where(hit_ok, cid, 0)
    best = np.zeros(N_CLUSTERS, f32)
    np.maximum.at(best, cid_eff, q32)
    idx = np.full(N_CLUSTERS, N_HITS, np.int64)
    ismax = (q32 == best[cid_eff]) & (cid_eff > 0)
    np.minimum.at(idx, cid_eff[ismax], np.nonzero(ismax)[0])
    alphas = np.where(idx[1:] < N_HITS, idx[1:], 0)      # [1023]

    # device operands: bf16-quantized hits + condensation points
    xq = x.astype(_BF16)                                 # [N, 32]
    xqf = xq.astype(f32)
    r2q = np.einsum('nd,nd->n', xqf, xqf).astype(f32)
    r2b = r2q.astype(_BF16)

    NPC = NPAIR * 256                                    # padded rows/core (6400)
    X34 = np.zeros((NCORES * NPC, CDIM), f32)
    real = np.zeros(NCORES * NPC, bool)
    for c in range(NCORES):
        real[c * NPC:c * NPC + N_PER] = True
    X34[real, :D_EMB] = xqf
    X34[real, D_EMB] = r2b.astype(f32)
    X34[:, D_EMB + 1] = 1.0
    X34 = X34.astype(_BF16)

    Y34 = np.zeros((KP, CDIM), f32)
    Y34[:N_OBJ, :D_EMB] = -2.0 * xqf[alphas]
    Y34[:N_OBJ, D_EMB] = 1.0
    Y34[:N_OBJ, D_EMB + 1] = r2b[alphas].astype(f32)
    Y34[N_OBJ] = 0.0
    Y34[N_OBJ, D_EMB] = 1.0
    Y34[N_OBJ, D_EMB + 1] = 1e4                          # dummy far column
    Y34 = Y34.astype(_BF16)
    xkt = np.zeros((128, KP), _BF16)
    xkt[0:CDIM] = Y34.T
    xkt[64:64 + CDIM] = Y34.T

    in_maps = []
    for c in range(NCORES):
        A = X34[c * NPC:(c + 1) * NPC].reshape(2 * NPAIR, 128, CDIM
                                               ).transpose(0, 2, 1)
        xt_c = np.zeros((128, NPAIR * 128), _BF16)
        xt_c[0:CDIM] = A[0::2].transpose(1, 0, 2).reshape(CDIM, NPAIR * 128)
        xt_c[64:64 + CDIM] = A[1::2].transpose(1, 0, 2).reshape(
            CDIM, NPAIR * 128)
        in_maps.append({"xt": np.ascontiguousarray(xt_c), "xkt": xkt})

    aux = dict(q32=q32, hit_ok=hit_ok, cid=cid, beta=beta, x=x,
               alphas=alphas)
    return in_maps, aux


# ---------------- host finish ----------------
def _finish(results, aux):
    f64 = np.float64
    q32, alphas = aux["q32"], aux["alphas"]
    hit_ok, cid, beta, x = aux["hit_ok"], aux["cid"], aux["beta"], aux["x"]

    q = q32.astype(f64)
    x64 = x.astype(f64)
    xk64 = x64[alphas]                                   # [1023, 32]
    qk = q[alphas]

    dve_tiles, act_tiles = _tile_split()
    nv = len(dve_tiles)

    # ---- gather flagged hits from the detectors ----
    flagged = set()
    for c in range(NCORES):
        det = np.asarray(results[c]["det_out"], f64)     # [128, nv+na]
        fl = np.zeros((128, NT), bool)
        fl[:, dve_tiles] = det[:, :nv] < THR - 0.5
        fl[:, act_tiles] = det[:, nv:] > 0.45
        rr, tt = np.nonzero(fl)
        for r, t in zip(rr, tt):
            n = t * 128 + r
            if n < N_PER:
                flagged.add(c * N_PER + n)
    flagged = np.fromiter(sorted(flagged), dtype=np.int64,
                          count=len(flagged))

    # safety: the detector's bf16 error margin assumes moderate |x|; the
    # dominant term is the bf16 rounding of |x|^2, so bound that directly
    if (not np.isfinite(x).all()) or \
            float(np.einsum('nd,nd->n', x64, x64).max()) > 200.0:
        flagged = np.arange(N_HITS, dtype=np.int64)
    if os.environ.get("COND_KERNEL_DEBUG", "0") == "1":
        print(f"[kernel] flagged rows: {len(flagged)}")

    # ---- v_rep: exact fp64 over flagged rows only ----
    v_rep_num = 0.0
    if len(flagged):
        xf = x64[flagged]
        d2 = (np.einsum('nd,nd->n', xf, xf)[:, None]
              + np.einsum('kd,kd->k', xk64, xk64)[None, :]
              - 2.0 * (xf @ xk64.T))
        dist = np.sqrt(np.maximum(d2, 1e-12))
        att = (cid[flagged][:, None] == np.arange(1, N_CLUSTERS)[None, :]) \
            & hit_ok[flagged][:, None]
        rep = (~att) & (dist < 1.0)
        qw = q[flagged][:, None] * qk[None, :]
        v_rep_num = float(np.sum(qw * (1.0 - dist) * rep))

    # ---- v_att: exact fp64 on the attractive pairs ----
    att_idx = np.nonzero(hit_ok & (cid > 0))[0]
    kk = cid[att_idx] - 1
    diff = x64[att_idx] - xk64[kk]
    d2a = np.maximum(np.einsum('nd,nd->n', diff, diff), 1e-12)
    v_att_num = float(np.sum(q[att_idx] * qk[kk] * d2a))

    n_hits_oi = float(hit_ok.sum())
    norm_att = EPS + n_hits_oi - N_OBJ
    norm_rep = EPS + (N_OBJ - 1) * N_HITS
    v_att = v_att_num / norm_att
    v_rep = v_rep_num / norm_rep

    noise_mask = (cid <= 0)
    l_noise = float(beta[noise_mask].sum()) / max(float(noise_mask.sum()), 1.0)
    l_coward = float(np.mean(1.0 - beta[alphas]))

    total = v_att + LW_REP * v_rep + LW_NOISE * l_noise + LW_COWARD * l_coward
    return np.asarray(total, dtype=np.float32)


# ---------------- execution backends ----------------
def _run_sim(nc, in_maps):
    from concourse.bass_interp import CoreSim
    results = []
    for m in in_maps:
        sim = CoreSim(nc)
        for k, v in m.items():
            sim.tensor(k)[:] = v
        sim.simulate()
        results.append({k: np.array(sim.tensor(k))
                        for k in ("det_out",)})
    return results


def _ensure_ntff_hook():
    """Register the axon NTFF profiling hook if the antenv shim lacks it."""
    import sys
    import types
    try:
        from antenv.axon_hooks import get_axon_ntff_profile_hook  # noqa: F401
        return
    except ImportError:
        pass
    from trn_agent_boot.trn_boot import _ntff_profile_via_ctypes
    hook = _ntff_profile_via_ctypes("/opt/axon/libaxon_pjrt.so")
    mod = types.ModuleType("antenv.axon_hooks")
    _h = [hook]
    mod.set_axon_ntff_profile_hook = lambda h: _h.__setitem__(0, h)
    mod.get_axon_ntff_profile_hook = lambda: _h[0]
    sys.modules["antenv.axon_hooks"] = mod
    import antenv
    antenv.axon_hooks = mod


def _run_hw(nc, in_maps, trace=False):
    import tempfile
    from concourse.bass_utils import run_bass_kernel_spmd
    core_ids = list(range(NCORES))
    if trace:
        try:
            _ensure_ntff_hook()
            tmpdir = tempfile.mkdtemp(prefix="cond_trace_")
            res = run_bass_kernel_spmd(nc, in_maps, core_ids, trace=True,
                                       tmpdir=tmpdir)
            _STATE["last_exec_time_ns"] = res.exec_time_ns
            _STATE["last_trace_dir"] = tmpdir
            _STATE["last_profile_json"] = res.profile_json
            return res.results
        except Exception:
            import traceback
            traceback.print_exc()
            print("[kernel] traced run failed; retrying without trace")
    res = run_bass_kernel_spmd(nc, in_maps, core_ids, trace=False)
    _STATE["last_exec_time_ns"] = res.exec_time_ns
    return res.results


def kernel(beta, x, pt, eta, reconstructable, cluster_ids, n_clusters=None,
           **_ignored):
    in_maps, aux = _prep(beta, x, pt, eta, reconstructable, cluster_ids)
    nc = _get_module()
    if os.environ.get("COND_KERNEL_SIM", "0") == "1":
        results = _run_sim(nc, in_maps)
    else:
        results = _run_hw(nc, in_maps,
                          trace=os.environ.get("COND_KERNEL_TRACE", "0") == "1")
    return _finish(results, aux)
